# revision 34
# baseline (speedup 1.0000x reference)
"""Conv2d(128->256, 3x3, pad 1) with LoRA (rank 8) — Trainium2 Bass kernel.

Strategy:
  - Data-parallel over batch: 16 images -> 2 per core x 8 cores. Conv weights
    and LoRA A/B replicated.
  - LoRA folds into the conv weight (conv is linear in weights):
        W_eff = W + (alpha/rank) * (B @ A).reshape(C_OUT, C_IN, 3, 3)
    computed on-device with 9 tiny PE matmuls + fused DVE add.
  - The 3x3 conv itself = 9 shifted matmuls accumulating in PSUM:
        out[co, pix] += W_eff[co, :, kh, kw]^T @ x_shift[ci, pix]
    with K = C_IN = 128 (partition dim), M = 128 (co block), N = 512
    (8 image rows x 64 cols) in bf16 (full PE rate + FWL weight loads).
  - Host-side prep is layout only (zero-padding + transposes), no arithmetic.
"""

import numpy as np

import concourse.bass as bass
import concourse.tile as tile
from concourse.tile import add_dep_helper
from concourse import bacc, mybir
from concourse.bass_utils import run_bass_kernel_spmd

N_CORES = 8
B, C_IN, H, W_DIM = 16, 128, 64, 64
C_OUT = 256
RANK = 8
SCALING = 2.0  # alpha/rank = 16/8
HP, WP = H + 2, W_DIM + 2  # zero-padded image dims
B_LOC = B // N_CORES  # images per core
NPIX = H * W_DIM  # 4096
ROWS_PER_TILE = 8  # output rows per matmul group -> N = 8*64 = 512
N_RG = H // ROWS_PER_TILE  # 8 row groups

F32 = mybir.dt.float32
F32R = mybir.dt.float32r
BF16 = mybir.dt.bfloat16
IDENT = mybir.ActivationFunctionType.Identity


def _build_nc():
    nc = bacc.Bacc(
        "TRN2",
        target_bir_lowering=False,
        debug=False,
        num_devices=N_CORES,
    )

    xp = nc.dram_tensor("xp", [B_LOC, C_IN, HP * WP], F32, kind="ExternalInput").ap()
    wt = nc.dram_tensor("wt", [C_IN, 9 * C_OUT], F32, kind="ExternalInput").ap()
    at = nc.dram_tensor("at", [RANK, 9 * C_IN], F32, kind="ExternalInput").ap()
    bt = nc.dram_tensor("bt", [RANK, C_OUT], F32, kind="ExternalInput").ap()
    bv = nc.dram_tensor("bv", [128, 2], F32, kind="ExternalInput").ap()
    out = nc.dram_tensor("out", [B_LOC, C_OUT, NPIX], F32, kind="ExternalOutput").ap()

    with tile.TileContext(nc) as tc:
        with (
            tc.tile_pool(name="persist", bufs=1) as persist,
            tc.tile_pool(name="outp", bufs=4) as outp,
            tc.tile_pool(name="psum", bufs=7, space="PSUM") as psum,
        ):
            # --- persistent SBUF tiles -------------------------------------
            # f32r operands must be produced by a rounding compute op (BIR
            # verifier rule), so x is staged f32 then DVE-converted to f32r.
            x_sb = [
                persist.tile([C_IN, HP * WP], F32, name=f"x_sb{i}")
                for i in range(B_LOC)
            ]
            x_sbr = [
                persist.tile([C_IN, HP * WP], BF16, name=f"x_sbr{i}")
                for i in range(B_LOC)
            ]
            wt_sb = persist.tile([C_IN, 9 * C_OUT], F32, name="wt_sb")
            weff = persist.tile([C_IN, 9 * C_OUT], BF16, name="weff")
            at_sb = persist.tile([RANK, 9 * C_IN], F32, name="at_sb")
            bt_sb = persist.tile([RANK, C_OUT], F32, name="bt_sb")
            at_sbr = persist.tile([RANK, 9 * C_IN], BF16, name="at_sbr")
            bt_sbr = persist.tile([RANK, C_OUT], BF16, name="bt_sbr")
            b_sb = persist.tile([128, 2], F32, name="b_sb")

            # --- PE warm-up ------------------------------------------------
            # The HAM clock gate holds the PE at 1.2 GHz until it has been
            # busy ~3.4us. Dummy matmuls on a zeroed scratch tile have no DMA
            # dependencies, so they warm the PE during the input prefetch.
            warm_sb = persist.tile([128, 512], F32, name="warm_sb")
            nc.gpsimd.memset(warm_sb[:], 0.0)
            warm_ps = psum.tile([128, 512], F32, tag="warm", bufs=1, name="warm_ps")
            for _ in range(3):
                nc.tensor.matmul(
                    warm_ps[:], warm_sb[:, :128], warm_sb[:], start=True, stop=True
                )

            # --- input DMAs ------------------------------------------------
            # Three DMA paths run in parallel (FIFO order per queue):
            #   sync HWDGE:   tiny LoRA operands, x0c1, wt_h0, x0c3, outs
            #   scalar HWDGE: x0c0, wt_h1, x0c2, outs
            #   gpsimd SWDGE: image 1 (slow first-byte, needed only ~45us in)
            # This gets at/bt in first (LoRA matmuls), image-0 chunks + the
            # weight halves land by ~15us, and keeps the HW queues free of
            # the 2.2MB image-1 traffic.
            qs = [nc.sync, nc.scalar]
            N_CHUNK = 6
            csz = (HP * WP + N_CHUNK - 1) // N_CHUNK
            chunks = [
                (i, c * csz, min((c + 1) * csz, HP * WP))
                for i in range(B_LOC)
                for c in range(N_CHUNK)
            ]

            def xdma(eng, i, c):
                lo, hi = c * csz, min((c + 1) * csz, HP * WP)
                eng.dma_start(x_sb[i][:, lo:hi], xp[i, :, lo:hi])

            hw = (9 * C_OUT) // 2
            # Startup is HBM-bandwidth constrained: prioritize exactly the
            # bytes the first conv groups need — W (for the weff fold) and
            # the first small x0 chunk — one weight half + interleaved x0
            # chunks per HW queue. Bias goes via SWDGE (128 tiny descriptors
            # would stall a HW queue head ~6us; gpsimd is idle).
            nc.gpsimd.dma_start(b_sb[:], bv)
            nc.sync.dma_start(at_sb[:], at)
            nc.sync.dma_start(bt_sb[:], bt)
            xdma(nc.scalar, 0, 0)
            # wt in quarters, alternating queues, so the weff adds (and with
            # them conv group 0) start as soon as the first quarter lands.
            wq = (9 * C_OUT) // 4
            for q in range(4):
                lo, hi = q * wq, min((q + 1) * wq, 9 * C_OUT)
                qs[q % 2].dma_start(wt_sb[:, lo:hi], wt[:, lo:hi])
            for c in range(1, N_CHUNK):
                xdma(qs[c % 2], 0, c)
            for c in range(N_CHUNK):
                xdma(qs[c % 2], 1, c)

            # --- fold LoRA into the conv weight ----------------------------
            # weff[:, k*256+co] = wt[:, k*256+co] + 2 * (A_k^T @ B^T)[ci, co]
            # (plain fp32 matmuls: tiny, and they extend the PE warm-up)
            #
            # The DVE stream is FIFO and the scheduler's DMA-latency model is
            # optimistic, so the early-critical DVE ops are put in an explicit
            # total order (each chained to the previous): cast(x0c0), the 9
            # weff adds (conv group 0 starts progressively off add#0), then
            # the remaining x0 casts. A mis-ordered cast would block the adds
            # — and all conv matmuls — behind a later x-chunk DMA.
            def chain(inst, prev, why):
                if prev is not None:
                    add_dep_helper(inst.ins, prev.ins, sync=False, reason=why)
                return inst

            def cast_chunk(i, lo, hi, prev):
                c = nc.vector.tensor_copy(x_sbr[i][:, lo:hi], x_sb[i][:, lo:hi])
                return chain(c, prev, "DVE prep total order")

            # tiny bf16 casts of the LoRA operands head the chain: bf16 LoRA
            # matmuls are 4x shorter on the PE FIFO ahead of the conv (their
            # rounding is on the already-small LoRA term — negligible).
            link = chain(nc.vector.tensor_copy(at_sbr[:], at_sb[:]), None, "")
            link = chain(
                nc.vector.tensor_copy(bt_sbr[:], bt_sb[:]), link, "DVE prep order"
            )
            i0, lo0, hi0 = chunks[0]
            link = cast_chunk(i0, lo0, hi0, link)
            for k in range(9):
                lps = psum.tile([128, C_OUT], F32, tag="ps", name=f"lps{k}")
                nc.tensor.matmul(
                    lps[:],
                    at_sbr[:, k * 128 : (k + 1) * 128],
                    bt_sbr[:],
                    start=True,
                    stop=True,
                )
                link = chain(
                    nc.vector.scalar_tensor_tensor(
                        weff[:, k * C_OUT : (k + 1) * C_OUT],
                        lps[:],
                        SCALING,
                        wt_sb[:, k * C_OUT : (k + 1) * C_OUT],
                        op0=mybir.AluOpType.mult,
                        op1=mybir.AluOpType.add,
                    ),
                    link,
                    "DVE prep total order",
                )
            for i, lo, hi in chunks[1:N_CHUNK]:
                link = cast_chunk(i, lo, hi, link)
            # x1 casts: after the whole prep chain, exact order left to the
            # scheduler (their data arrives ~25us; needed ~50us).
            for i, lo, hi in chunks[N_CHUNK:]:
                cast_chunk(i, lo, hi, link)

            # --- the conv: 9 accumulating shift-matmuls per output tile ----
            for img in range(B_LOC):
                x_r = x_sbr[img][:].rearrange("p (h w) -> p h w", w=WP)
                for cb in range(2):
                    for rg in range(N_RG):
                        ps = psum.tile([128, 512], F32, tag="ps", name=f"ps{img}_{cb}_{rg}")
                        h0 = rg * ROWS_PER_TILE
                        for k in range(9):
                            dh, dw = k // 3 - 1, k % 3 - 1
                            rhs = x_r[
                                :,
                                h0 + 1 + dh : h0 + 1 + dh + ROWS_PER_TILE,
                                1 + dw : 65 + dw,
                            ]
                            lhsT = weff[:, k * 256 + cb * 128 : k * 256 + cb * 128 + 128]
                            nc.tensor.matmul(
                                ps[:],
                                lhsT,
                                rhs,
                                start=(k == 0),
                                stop=(k == 8),
                            )
                        o = outp.tile([128, 512], F32, tag="o", name=f"o{img}_{cb}_{rg}")
                        ti = (img * 2 + cb) * N_RG + rg
                        # Alternate the PSUM->SBUF bias-add between ACT and DVE
                        # so neither engine limits the drain of PSUM banks.
                        if ti % 2 == 0:
                            nc.scalar.activation(
                                o[:], ps[:], IDENT, bias=b_sb[:, cb : cb + 1]
                            )
                        else:
                            nc.vector.tensor_scalar_add(
                                o[:], ps[:], b_sb[:, cb : cb + 1]
                            )
                        dst = out[
                            img, cb * 128 : (cb + 1) * 128, rg * 512 : (rg + 1) * 512
                        ]
                        if ti >= 30:
                            # split the final tiles across both queues to
                            # shorten the drain tail
                            qs[0].dma_start(dst[:, :256], o[:, :256])
                            qs[1].dma_start(dst[:, 256:], o[:, 256:])
                        else:
                            qs[ti % 2].dma_start(dst, o[:])

    nc.compile()
    return nc


_NC_CACHE = None


def _get_nc():
    global _NC_CACHE
    if _NC_CACHE is None:
        _NC_CACHE = _build_nc()
    return _NC_CACHE


def _host_prep(x, W, b, lora_A, lora_B):
    """Layout-only host prep (pad + transpose); no arithmetic."""
    x = np.ascontiguousarray(x, dtype=np.float32)
    xp_all = np.zeros((B, C_IN, HP, WP), dtype=np.float32)
    xp_all[:, :, 1 : H + 1, 1 : W_DIM + 1] = x
    xp_all = xp_all.reshape(B, C_IN, HP * WP)

    # [co, ci, kh, kw] -> [ci, k, co]
    wt = np.ascontiguousarray(
        np.asarray(W, dtype=np.float32).reshape(C_OUT, C_IN, 9).transpose(1, 2, 0)
    ).reshape(C_IN, 9 * C_OUT)
    # [r, ci*9+k] -> [r, k, ci]
    at = np.ascontiguousarray(
        np.asarray(lora_A, dtype=np.float32).reshape(RANK, C_IN, 9).transpose(0, 2, 1)
    ).reshape(RANK, 9 * C_IN)
    # [co, r] -> [r, co]
    bt = np.ascontiguousarray(np.asarray(lora_B, dtype=np.float32).T)
    # [256] -> [128, 2]: bv[p, cb] = b[cb*128 + p]
    bv = np.ascontiguousarray(np.asarray(b, dtype=np.float32).reshape(2, 128).T)
    return xp_all, wt, at, bt, bv


def run(x, W, b, lora_A, lora_B, trace=False):
    """Run the kernel on 8 cores; returns (full_output, BassKernelResults)."""
    xp_all, wt, at, bt, bv = _host_prep(x, W, b, lora_A, lora_B)
    nc = _get_nc()
    in_maps = []
    for c in range(N_CORES):
        in_maps.append(
            {
                "xp": np.ascontiguousarray(xp_all[c * B_LOC : (c + 1) * B_LOC]),
                "wt": wt,
                "at": at,
                "bt": bt,
                "bv": bv,
            }
        )
    res = run_bass_kernel_spmd(
        nc, in_maps, core_ids=list(range(N_CORES)), trace=trace
    )
    out = np.concatenate([r["out"] for r in res.results], axis=0)
    return out.reshape(B, C_OUT, H, W_DIM), res


def kernel(x, W, b, lora_A, lora_B):
    out, _ = run(x, W, b, lora_A, lora_B, trace=False)
    return out
